# revision 25
# baseline (speedup 1.0000x reference)
"""Trainium2 Bass kernel: single-layer tanh RNN (T=512, B=64, IN=H=1024).

Strategy: data-parallel over batch (8 rows/core on 8 cores, no collectives).
Per core, per step t: PSUM bank (alternating pa/pb) accumulates
  xp[t]  (injected via identity-matmul from a precomputed input projection)
  + h_t @ W_hh.T  (8 K-chunks x 4 column-tile groups, bf16 weights/state)
then ScalarE applies tanh (bf16 copy feeds the recurrence, f32 copy is DMA'd
out), and DVE's 32x32 block-transpose converts the state back into lhsT
layout for the next step. The input GEMM (x @ W_ih.T + bias, f32-accurate
bias via an f32 ones-row matmul) runs in the PE tail gaps two 16-step chunks
ahead of the recurrence.

All matmuls are 128x32 column-tiled: group g owns PSUM partitions
[32g, 32g+32); batch rows live at [32g, 32g+8). Garbage partitions are
zeroed once by prologue zero-matmuls and never rewritten.

Layouts (per core, hidden index n = 256*g + n', contraction k re-indexed as
k = 256*I + 32*J + a so that the DVE block transpose of the state lands
directly in lhsT order):
  w    [128, 8192] bf16: w[32I+a, 1024J+n] = W_hh.T[256I+32J+a, n]
  wih  [128, 8192] bf16: wih[p, 1024i+n]   = W_ih.T[128i+p, n]
  xt   [IN, T*8]   bf16: xt[i, 8t+b]       = input[t, b, i]
  d0   [128, 256]  bf16: d0[32I+a, 32J+c]  = h0[c, 256I+32J+a] (c<8 else 0)
  out  [T, 4, 8, 256] f32: out[t, g, b, n'] = h_{t+1}[b, 256g+n']
"""

import os
import sys

for _p in ("/opt/trn_rl_repo",):
    if os.path.isdir(_p) and _p not in sys.path:
        sys.path.insert(0, _p)

from contextlib import ExitStack

import ml_dtypes
import numpy as np

import concourse.bass as bass
import concourse.tile as tile
from concourse import mybir
from concourse.bacc import Bacc
from concourse.bass_utils import run_bass_kernel_spmd

T_FULL, B, IN, H = 512, 64, 1024, 1024
NCORES = 8
MB = B // NCORES  # batch rows per core

F32 = mybir.dt.float32
BF16 = mybir.dt.bfloat16
TANH = mybir.ActivationFunctionType.Tanh
NPBF16 = ml_dtypes.bfloat16


def build_nc(T=T_FULL):
    assert T % 16 == 0
    NCH = T // 16            # 16-step chunks of the input projection
    NBUF = min(5, NCH)       # xt chunks resident in SBUF

    # Bacc (not raw Bass): its compile() runs generate_event_semaphores,
    # which legalizes TRN2's 1-sync-wait-per-instruction limit by splitting
    # excess waits onto EventSemaphore instructions.
    nc = Bacc()
    xt_d = nc.declare_dram_parameter("xt", [IN, T * MB], F32, isOutput=False)
    w_d = nc.declare_dram_parameter("w", [128, 8 * H], BF16, isOutput=False)
    # wih carries W_ih.T in cols [0, 8H) and the bias broadcast to all 128
    # partitions in cols [8H, 9H): one DMA -> one semaphore for both.
    wih_d = nc.declare_dram_parameter("wih", [128, 9 * H], F32, isOutput=False)
    d0_d = nc.declare_dram_parameter("d0", [128, 256], BF16, isOutput=False)
    id_d = nc.declare_dram_parameter("ident", [128, 128], F32, isOutput=False)
    out_d = nc.declare_dram_parameter("out", [T, 4, MB, 256], F32, isOutput=True)

    with tile.TileContext(nc) as tc, ExitStack() as ctx:
        sb = ctx.enter_context(tc.tile_pool(name="sb", bufs=1))
        ps = ctx.enter_context(
            tc.tile_pool(name="ps", bufs=1, space=bass.MemorySpace.PSUM)
        )

        w_sb = sb.tile([128, 8 * H], BF16, tag="w")
        wih_sb = sb.tile([128, 9 * H], F32, tag="wih")
        id_sb = sb.tile([128, 128], F32, tag="id")
        z256 = sb.tile([128, 256], BF16, tag="z256")
        D = [sb.tile([128, 256], BF16, tag="D", bufs=2, name=f"Dt{i}") for i in range(2)]

        xtc_tiles = {}

        def issue_xt_dma(c):
            # stream xp chunk c's input slice: xt columns [128c, 128c+128)
            # SP HW-DGE DMACopy has a single sync-wait slot; recycled buffers
            # need a ring wait AND a WAR-on-PE wait, so route those through
            # the Pool SW-DGE queue, issuing tile 7 first: it carries the max
            # PE threshold (PE reads it last), so queue-order subsumption
            # strips the PE wait from the other 7 DMAs.
            tiles = [
                sb.tile([128, 128], F32, tag="xtc", bufs=8 * NBUF, name=f"xtc{c}_{i}")
                for i in range(8)
            ]
            xtc_tiles[c] = tiles
            eng = nc.gpsimd if c >= NBUF else nc.sync
            for i in (7, 0, 1, 2, 3, 4, 5, 6):
                eng.dma_start(
                    tiles[i][:],
                    xt_d[128 * i : 128 * i + 128, 128 * c : 128 * c + 128],
                )

        pa = ps.tile([128, 256], F32, tag="pa")
        pb = ps.tile([128, 256], F32, tag="pb")
        pin = [ps.tile([128, 512], F32, tag="pin", bufs=2, name=f"pin{i}") for i in range(2)]
        banks = [pa, pb]

        nc.gpsimd.memset(z256[:], 0.0)

        io = nc.sync
        io.dma_start(id_sb[:], id_d[:])
        io.dma_start(D[0][:], d0_d[:])
        io.dma_start(wih_sb[:], wih_d[:])
        issue_xt_dma(0)
        io.dma_start(w_sb[:], w_d[:])
        for c in range(1, min(3, NCH)):
            issue_xt_dma(c)

        # Zero all PSUM partitions of both recurrence banks once. Later MMs
        # only write rows [32g, 32g+MB); the tanh reads all 128 partitions.
        for bank in banks:
            for g in range(4):
                nc.tensor.matmul(
                    bank[32 * g : 32 * g + 32, :],
                    z256[:, 0:32],
                    z256[:],
                    start=True,
                    stop=True,
                    tile_position=(0, 32 * g),
                        skip_group_check=True,
                )

        # PE Matmult instructions have a single HW sync-wait slot. Pre-warm
        # PE with one dummy matmul per prologue DMA semaphore (id, wih+bias,
        # d0) so every later matmul needs at most one new wait. w and the
        # xtc tiles are first consumed alone, so they need no pre-warm.
        pscr = ps.tile([32, 32], F32, tag="pscr")
        warm = [
            (id_sb[:, 0:32], id_sb[:, 0:32]),
            (id_sb[:, 0:32], wih_sb[:, 0:32]),
            (D[0][:, 0:32], z256[:, 0:32]),
        ]
        for lhsT, rhs in warm:
            nc.tensor.matmul(
                pscr[:],
                lhsT,
                rhs,
                start=True,
                stop=True,
                tile_position=(0, 0),
                skip_group_check=True,
            )

        xp_tiles = {}

        def make_gemm(c):
            # ops to compute xp chunk c: [128 rows = 16 steps x 8 batch, H] f32
            xp_c = sb.tile([128, H], F32, tag="xp", bufs=4, name=f"xp{c}")
            xp_tiles[c] = xp_c
            xtc = xtc_tiles[c]
            ops = []
            for nh in range(2):
                p = pin[nh]

                def em_bias(nh=nh, p=p):
                    # bias broadcast: all 128 partitions of wih_sb[:, 8H:9H]
                    # hold the same bias row, and identity columns have unit
                    # column-sums, so out[c, n] = bias[n] for every c.
                    for g in range(4):
                        nc.tensor.matmul(
                            p[32 * g : 32 * g + 32, :],
                            id_sb[:, 0:32],
                            wih_sb[:, 8 * H + 512 * nh : 8 * H + 512 * nh + 512],
                            start=True,
                            stop=False,
                            tile_position=(0, 32 * g),
                        skip_group_check=True,
                        )

                ops.append(em_bias)
                for i in range(8):

                    def em_mm(i=i, nh=nh, p=p, xtc=xtc):
                        for g in range(4):
                            nc.tensor.matmul(
                                p[32 * g : 32 * g + 32, :],
                                xtc[i][:, 32 * g : 32 * g + 32],
                                wih_sb[:, 1024 * i + 512 * nh : 1024 * i + 512 * nh + 512],
                                start=False,
                                stop=(i == 7),
                                tile_position=(0, 32 * g),
                        skip_group_check=True,
                            )

                    ops.append(em_mm)

                def em_drain(nh=nh, p=p, xp_c=xp_c):
                    nc.scalar.copy(xp_c[:, 512 * nh : 512 * nh + 512], p[:, :])

                ops.append(em_drain)
            return ops

        def inject(t):
            bank = banks[t % 2]
            xp_c = xp_tiles[t // 16]
            tl = t % 16
            for g in range(4):
                nc.tensor.matmul(
                    bank[32 * g : 32 * g + MB, :],
                    id_sb[:, MB * tl : MB * tl + MB],
                    xp_c[:, 256 * g : 256 * g + 256],
                    start=True,
                    stop=False,
                    tile_position=(0, 32 * g),
                        skip_group_check=True,
                )

        for op in make_gemm(0):
            op()
        if NCH > 1:
            for op in make_gemm(1):
                op()
        inject(0)

        pending = []
        for t in range(T):
            bank = banks[t % 2]
            Dt = D[t % 2]
            for J in range(8):
                for g in range(4):
                    nc.tensor.matmul(
                        bank[32 * g : 32 * g + MB, :],
                        Dt[:, 32 * J : 32 * J + MB],
                        w_sb[:, 1024 * J + 256 * g : 1024 * J + 256 * g + 256],
                        start=False,
                        stop=(J == 7),
                        tile_position=(0, 32 * g),
                        skip_group_check=True,
                    )
            sb16 = sb.tile([128, 256], BF16, tag="sb16", bufs=3, name="sb16")
            sf = sb.tile([128, 256], F32, tag="sf", bufs=3, name="sf")
            nc.scalar.activation(sb16[:], bank[:], TANH)
            nc.scalar.activation(sf[:], bank[:], TANH)
            if t + 1 < T:
                nc.vector.transpose(D[(t + 1) % 2][:], sb16[:])
            for g in range(4):
                nc.gpsimd.dma_start(out_d[t, g], sf[32 * g : 32 * g + MB, :])
            if t + 1 < T:
                inject(t + 1)
            if t % 16 == 0 and t // 16 + 3 < NCH:
                issue_xt_dma(t // 16 + 3)
            c_next = t // 16 + 2
            if t % 16 == 0 and c_next < NCH:
                pending.extend(make_gemm(c_next))
            for _ in range(2):
                if pending:
                    pending.pop(0)()
    return nc


def _prep_shared(W_ih, W_hh, b_ih, b_hh):
    WT = np.ascontiguousarray(W_hh.T)
    w_host = np.ascontiguousarray(
        WT.reshape(4, 8, 32, H).transpose(0, 2, 1, 3).reshape(128, 8 * H)
    ).astype(NPBF16)
    wih_mat = (
        np.ascontiguousarray(W_ih.T).reshape(8, 128, H).transpose(1, 0, 2).reshape(128, 8 * H)
    )
    bias = (b_ih + b_hh).astype(np.float32).reshape(1, H)
    wih_host = np.ascontiguousarray(
        np.concatenate([wih_mat, np.broadcast_to(bias, (128, H))], axis=1)
    ).astype(np.float32)
    ident = np.eye(128, dtype=np.float32)
    return w_host, wih_host, ident


def _prep_core(input_np, h0, cid, T):
    sl = input_np[:, MB * cid : MB * cid + MB, :]  # [T, 8, IN]
    xt = np.ascontiguousarray(sl.reshape(T * MB, IN).T).astype(np.float32)  # [IN, T*8]
    h0c = h0[MB * cid : MB * cid + MB]  # [8, H]
    d0 = np.zeros((4, 32, 8, 32), np.float32)
    d0[:, :, :, 0:MB] = h0c.reshape(MB, 4, 8, 32).transpose(1, 3, 2, 0)
    d0 = np.ascontiguousarray(d0.reshape(128, 256)).astype(NPBF16)
    return xt, d0


def _run(input_np, hidden_init, W_ih, W_hh, b_ih, b_hh, T, run_kwargs=None):
    nc = build_nc(T)
    w_host, wih_host, ident = _prep_shared(W_ih, W_hh, b_ih, b_hh)
    h0 = hidden_init[0]
    in_maps = []
    for cid in range(NCORES):
        xt, d0 = _prep_core(input_np, h0, cid, T)
        in_maps.append(
            {
                "xt": xt,
                "w": w_host,
                "wih": wih_host,
                "d0": d0,
                "ident": ident,
            }
        )
    res = run_bass_kernel_spmd(nc, in_maps, list(range(NCORES)), **(run_kwargs or {}))
    outs = np.empty((T, B, H), np.float32)
    for cid in range(NCORES):
        r = np.asarray(res.results[cid]["out"])  # [T, 4, MB, 256]
        outs[:, MB * cid : MB * cid + MB, :] = r.transpose(0, 2, 1, 3).reshape(T, MB, H)
    return outs, res


def kernel(input, hidden_init, W_ih, W_hh, b_ih, b_hh):
    input = np.asarray(input, dtype=np.float32)
    hidden_init = np.asarray(hidden_init, dtype=np.float32)
    W_ih = np.asarray(W_ih, dtype=np.float32)
    W_hh = np.asarray(W_hh, dtype=np.float32)
    b_ih = np.asarray(b_ih, dtype=np.float32)
    b_hh = np.asarray(b_hh, dtype=np.float32)
    outs, _ = _run(input, hidden_init, W_ih, W_hh, b_ih, b_hh, T_FULL)
    return outs, outs[-1].copy()


# revision 26
# speedup vs baseline: 1.0209x; 1.0209x over previous
"""Trainium2 Bass kernel: single-layer tanh RNN (T=512, B=64, IN=H=1024).

Strategy: data-parallel over batch (8 rows/core on 8 cores, no collectives).
Per core, per step t: PSUM bank (alternating pa/pb) accumulates
  xp[t]  (injected via identity-matmul from a precomputed input projection)
  + h_t @ W_hh.T  (8 K-chunks x 4 column-tile groups, bf16 weights/state)
then ScalarE applies tanh (bf16 copy feeds the recurrence, f32 copy is DMA'd
out), and DVE's 32x32 block-transpose converts the state back into lhsT
layout for the next step. The input GEMM (x @ W_ih.T + bias, f32-accurate
bias via an f32 ones-row matmul) runs in the PE tail gaps two 16-step chunks
ahead of the recurrence.

All matmuls are 128x32 column-tiled: group g owns PSUM partitions
[32g, 32g+32); batch rows live at [32g, 32g+8). Garbage partitions are
zeroed once by prologue zero-matmuls and never rewritten.

Layouts (per core, hidden index n = 256*g + n', contraction k re-indexed as
k = 256*I + 32*J + a so that the DVE block transpose of the state lands
directly in lhsT order):
  w    [128, 8192] bf16: w[32I+a, 1024J+n] = W_hh.T[256I+32J+a, n]
  wih  [128, 8192] bf16: wih[p, 1024i+n]   = W_ih.T[128i+p, n]
  xt   [IN, T*8]   bf16: xt[i, 8t+b]       = input[t, b, i]
  d0   [128, 256]  bf16: d0[32I+a, 32J+c]  = h0[c, 256I+32J+a] (c<8 else 0)
  out  [T, 4, 8, 256] f32: out[t, g, b, n'] = h_{t+1}[b, 256g+n']
"""

import os
import sys

for _p in ("/opt/trn_rl_repo",):
    if os.path.isdir(_p) and _p not in sys.path:
        sys.path.insert(0, _p)

from contextlib import ExitStack

import ml_dtypes
import numpy as np

import concourse.bass as bass
import concourse.tile as tile
from concourse import mybir
from concourse.bacc import Bacc
from concourse.bass_utils import run_bass_kernel_spmd

T_FULL, B, IN, H = 512, 64, 1024, 1024
NCORES = 8
MB = B // NCORES  # batch rows per core

F32 = mybir.dt.float32
BF16 = mybir.dt.bfloat16
TANH = mybir.ActivationFunctionType.Tanh
NPBF16 = ml_dtypes.bfloat16


def build_nc(T=T_FULL):
    assert T % 16 == 0
    NCH = T // 16            # 16-step chunks of the input projection
    NBUF = min(5, NCH)       # xt chunks resident in SBUF

    # Bacc (not raw Bass): its compile() runs generate_event_semaphores,
    # which legalizes TRN2's 1-sync-wait-per-instruction limit by splitting
    # excess waits onto EventSemaphore instructions.
    nc = Bacc()
    xt_d = nc.declare_dram_parameter("xt", [IN, T * MB], F32, isOutput=False)
    w_d = nc.declare_dram_parameter("w", [128, 8 * H], BF16, isOutput=False)
    # wih carries W_ih.T in cols [0, 8H) and the bias broadcast to all 128
    # partitions in cols [8H, 9H): one DMA -> one semaphore for both.
    wih_d = nc.declare_dram_parameter("wih", [128, 9 * H], F32, isOutput=False)
    d0_d = nc.declare_dram_parameter("d0", [128, 256], BF16, isOutput=False)
    id_d = nc.declare_dram_parameter("ident", [128, 128], F32, isOutput=False)
    out_d = nc.declare_dram_parameter("out", [T, 4, MB, 256], F32, isOutput=True)

    with tile.TileContext(nc) as tc, ExitStack() as ctx:
        sb = ctx.enter_context(tc.tile_pool(name="sb", bufs=1))
        ps = ctx.enter_context(
            tc.tile_pool(name="ps", bufs=1, space=bass.MemorySpace.PSUM)
        )

        w_sb = sb.tile([128, 8 * H], BF16, tag="w")
        wih_sb = sb.tile([128, 9 * H], F32, tag="wih")
        id_sb = sb.tile([128, 128], F32, tag="id")
        z256 = sb.tile([128, 256], BF16, tag="z256")
        D = [sb.tile([128, 256], BF16, tag="D", bufs=2, name=f"Dt{i}") for i in range(2)]

        xtc_tiles = {}

        def issue_xt_dma(c):
            # stream xp chunk c's input slice: xt columns [128c, 128c+128)
            # SP HW-DGE DMACopy has a single sync-wait slot; recycled buffers
            # need a ring wait AND a WAR-on-PE wait, so route those through
            # the Pool SW-DGE queue, issuing tile 7 first: it carries the max
            # PE threshold (PE reads it last), so queue-order subsumption
            # strips the PE wait from the other 7 DMAs.
            tiles = [
                sb.tile([128, 128], F32, tag="xtc", bufs=8 * NBUF, name=f"xtc{c}_{i}")
                for i in range(8)
            ]
            xtc_tiles[c] = tiles
            eng = nc.gpsimd if c >= NBUF else nc.sync
            for i in (7, 0, 1, 2, 3, 4, 5, 6):
                eng.dma_start(
                    tiles[i][:],
                    xt_d[128 * i : 128 * i + 128, 128 * c : 128 * c + 128],
                )

        pa = ps.tile([128, 256], F32, tag="pa")
        pb = ps.tile([128, 256], F32, tag="pb")
        pin = [ps.tile([128, 512], F32, tag="pin", bufs=2, name=f"pin{i}") for i in range(2)]
        banks = [pa, pb]

        nc.gpsimd.memset(z256[:], 0.0)

        io = nc.sync
        io.dma_start(id_sb[:], id_d[:])
        io.dma_start(D[0][:], d0_d[:])
        io.dma_start(wih_sb[:], wih_d[:])
        issue_xt_dma(0)
        io.dma_start(w_sb[:], w_d[:])
        for c in range(1, min(3, NCH)):
            issue_xt_dma(c)

        # Zero all PSUM partitions of both recurrence banks once. Later MMs
        # only write rows [32g, 32g+MB); the tanh reads all 128 partitions.
        for bank in banks:
            for g in range(4):
                nc.tensor.matmul(
                    bank[32 * g : 32 * g + 32, :],
                    z256[:, 0:32],
                    z256[:],
                    start=True,
                    stop=True,
                    tile_position=(0, 32 * g),
                        skip_group_check=True,
                )

        # PE Matmult instructions have a single HW sync-wait slot. Pre-warm
        # PE with one dummy matmul per prologue DMA semaphore (id, wih+bias,
        # d0) so every later matmul needs at most one new wait. w and the
        # xtc tiles are first consumed alone, so they need no pre-warm.
        pscr = ps.tile([32, 32], F32, tag="pscr")
        warm = [
            (id_sb[:, 0:32], id_sb[:, 0:32]),
            (id_sb[:, 0:32], wih_sb[:, 0:32]),
            (D[0][:, 0:32], z256[:, 0:32]),
        ]
        for lhsT, rhs in warm:
            nc.tensor.matmul(
                pscr[:],
                lhsT,
                rhs,
                start=True,
                stop=True,
                tile_position=(0, 0),
                skip_group_check=True,
            )

        xp_tiles = {}

        def make_gemm(c):
            # ops to compute xp chunk c: [128 rows = 16 steps x 8 batch, H] f32
            xp_c = sb.tile([128, H], F32, tag="xp", bufs=4, name=f"xp{c}")
            xp_tiles[c] = xp_c
            xtc = xtc_tiles[c]
            ops = []
            for nh in range(2):
                p = pin[nh]

                def em_bias(nh=nh, p=p):
                    # bias broadcast: all 128 partitions of wih_sb[:, 8H:9H]
                    # hold the same bias row, and identity columns have unit
                    # column-sums, so out[c, n] = bias[n] for every c.
                    for g in range(4):
                        nc.tensor.matmul(
                            p[32 * g : 32 * g + 32, :],
                            id_sb[:, 0:32],
                            wih_sb[:, 8 * H + 512 * nh : 8 * H + 512 * nh + 512],
                            start=True,
                            stop=False,
                            tile_position=(0, 32 * g),
                        skip_group_check=True,
                        )

                ops.append(em_bias)
                for i in range(8):

                    def em_mm(i=i, nh=nh, p=p, xtc=xtc):
                        for g in range(4):
                            nc.tensor.matmul(
                                p[32 * g : 32 * g + 32, :],
                                xtc[i][:, 32 * g : 32 * g + 32],
                                wih_sb[:, 1024 * i + 512 * nh : 1024 * i + 512 * nh + 512],
                                start=False,
                                stop=(i == 7),
                                tile_position=(0, 32 * g),
                        skip_group_check=True,
                            )

                    ops.append(em_mm)

                def em_drain(nh=nh, p=p, xp_c=xp_c):
                    nc.scalar.copy(xp_c[:, 512 * nh : 512 * nh + 512], p[:, :])

                ops.append(em_drain)
            return ops

        def inject(t):
            bank = banks[t % 2]
            xp_c = xp_tiles[t // 16]
            tl = t % 16
            for g in range(4):
                nc.tensor.matmul(
                    bank[32 * g : 32 * g + MB, :],
                    id_sb[:, MB * tl : MB * tl + MB],
                    xp_c[:, 256 * g : 256 * g + 256],
                    start=True,
                    stop=False,
                    tile_position=(0, 32 * g),
                        skip_group_check=True,
                )

        for op in make_gemm(0):
            op()
        if NCH > 1:
            for op in make_gemm(1):
                op()
        inject(0)

        pending = []
        for t in range(T):
            bank = banks[t % 2]
            Dt = D[t % 2]
            for J in range(8):
                for g in range(4):
                    nc.tensor.matmul(
                        bank[32 * g : 32 * g + MB, :],
                        Dt[:, 32 * J : 32 * J + MB],
                        w_sb[:, 1024 * J + 256 * g : 1024 * J + 256 * g + 256],
                        start=False,
                        stop=(J == 7),
                        tile_position=(0, 32 * g),
                        skip_group_check=True,
                    )
            sb16 = sb.tile([128, 256], BF16, tag="sb16", bufs=3, name="sb16")
            sf = sb.tile([128, 256], F32, tag="sf", bufs=3, name="sf")
            nc.scalar.activation(sb16[:], bank[:], TANH)
            nc.scalar.activation(sf[:], bank[:], TANH)
            if t + 1 < T:
                nc.vector.transpose(D[(t + 1) % 2][:], sb16[:])
            for g in range(4):
                nc.gpsimd.dma_start(out_d[t, g], sf[32 * g : 32 * g + MB, :])
            if t + 1 < T:
                inject(t + 1)
            if t % 16 == 0 and t // 16 + 3 < NCH:
                issue_xt_dma(t // 16 + 3)
            c_next = t // 16 + 2
            if t % 16 == 0 and c_next < NCH:
                pending.extend(make_gemm(c_next))
            for _ in range(2):
                if pending:
                    pending.pop(0)()
    return nc


def _prep_shared(W_ih, W_hh, b_ih, b_hh):
    WT = np.ascontiguousarray(W_hh.T)
    w_host = np.ascontiguousarray(
        WT.reshape(4, 8, 32, H).transpose(0, 2, 1, 3).reshape(128, 8 * H)
    ).astype(NPBF16)
    wih_mat = (
        np.ascontiguousarray(W_ih.T).reshape(8, 128, H).transpose(1, 0, 2).reshape(128, 8 * H)
    )
    bias = (b_ih + b_hh).astype(np.float32).reshape(1, H)
    wih_host = np.ascontiguousarray(
        np.concatenate([wih_mat, np.broadcast_to(bias, (128, H))], axis=1)
    ).astype(np.float32)
    ident = np.eye(128, dtype=np.float32)
    return w_host, wih_host, ident


def _prep_core(input_np, h0, cid, T):
    sl = input_np[:, MB * cid : MB * cid + MB, :]  # [T, 8, IN]
    xt = np.ascontiguousarray(sl.reshape(T * MB, IN).T).astype(np.float32)  # [IN, T*8]
    h0c = h0[MB * cid : MB * cid + MB]  # [8, H]
    d0 = np.zeros((4, 32, 8, 32), np.float32)
    d0[:, :, :, 0:MB] = h0c.reshape(MB, 4, 8, 32).transpose(1, 3, 2, 0)
    d0 = np.ascontiguousarray(d0.reshape(128, 256)).astype(NPBF16)
    return xt, d0


def _run(input_np, hidden_init, W_ih, W_hh, b_ih, b_hh, T, run_kwargs=None):
    nc = build_nc(T)
    nc.finalize()  # Bacc: runs compile() (reg alloc + wait legalization)
    w_host, wih_host, ident = _prep_shared(W_ih, W_hh, b_ih, b_hh)
    h0 = hidden_init[0]
    in_maps = []
    for cid in range(NCORES):
        xt, d0 = _prep_core(input_np, h0, cid, T)
        in_maps.append(
            {
                "xt": xt,
                "w": w_host,
                "wih": wih_host,
                "d0": d0,
                "ident": ident,
            }
        )
    res = run_bass_kernel_spmd(nc, in_maps, list(range(NCORES)), **(run_kwargs or {}))
    outs = np.empty((T, B, H), np.float32)
    for cid in range(NCORES):
        r = np.asarray(res.results[cid]["out"])  # [T, 4, MB, 256]
        outs[:, MB * cid : MB * cid + MB, :] = r.transpose(0, 2, 1, 3).reshape(T, MB, H)
    return outs, res


def kernel(input, hidden_init, W_ih, W_hh, b_ih, b_hh):
    input = np.asarray(input, dtype=np.float32)
    hidden_init = np.asarray(hidden_init, dtype=np.float32)
    W_ih = np.asarray(W_ih, dtype=np.float32)
    W_hh = np.asarray(W_hh, dtype=np.float32)
    b_ih = np.asarray(b_ih, dtype=np.float32)
    b_hh = np.asarray(b_hh, dtype=np.float32)
    outs, _ = _run(input, hidden_init, W_ih, W_hh, b_ih, b_hh, T_FULL)
    return outs, outs[-1].copy()
